# revision 1
# baseline (speedup 1.0000x reference)
"""Trainium2 Bass kernel for nn_LocalLoadBalancingLoss (v2).

loss = mean_b var_l(u) + 0.5 * mean_b max_l(u),
u[b,l] = (sum_{t: link(t)=l} pred[b,t] * dem[b, t//8]) / (cap[l] + 1e-8)

Strategy (pure data parallel over batch, 8 cores x 8192 rows):
  Row-paired tiles: one tile = 256 rows laid out [128p, 2rr, ...] with
  partition p holding DRAM rows {base+2p, base+2p+1}. This makes every
  DMA descriptor contiguous and >= 792 B (dem rows alone are 396 B,
  below the 512 B RMW threshold), and lets group DMAs move 2.5 MB per
  dma_start.

  Per 256-row tile (free width TW = 2*792 = 1584, padded to 1664):
    - DVE: tt(bf16) = pred * broadcast(dem)   (two ops, one per rr)
    - PE : 13x transpose of 128-wide tt chunks -> PSUM (bf16)
    - ACT: evacuate ttT PSUM -> SBUF (2 copies: 8-chunk + 5-chunk banks)
    - PE : 13x scatter matmul u[128, 2, 16] += ttT_c.T @ mask_c
           (mask one-hot built on host from tunnel_to_link; rows of the
            pad region are zero so garbage there is harmless)
  Per group of 4 tiles (1024 rows): u_ps [128, 4, 2, 16] fills one PSUM
  bank; scale by 1/cap (f32), fused DVE stats (sum/max/sum-of-squares)
  into [128, ng] accumulators.  Host: tiny final reduction across cores.
"""

from contextlib import ExitStack

import numpy as np

import concourse.bass as bass
import concourse.tile as tile
from concourse import mybir
from concourse.bass_utils import run_bass_kernel_spmd
from bass_rust import ScopedClock

N_CORES = 8
B, T, D, L = 65536, 792, 99, 16
ROWS = B // N_CORES  # 8192 rows per core
P = 128
RR = 2  # rows per partition
TROWS = P * RR  # 256 rows per tile
NT = ROWS // TROWS  # 32 tiles per core
TW = RR * T  # 1584 free elems per tile
TWPAD = 1664  # 13 * 128
NCH = TWPAD // P  # 13 chunks
NCA, NCB = 8, 5  # chunks in PSUM bank A / bank B
G = 4  # tiles per stats group (G*RR*L = 512 f32 = one PSUM bank)
NG = NT // G  # 8 groups
GROWS = G * TROWS  # 1024 rows per group

F32 = mybir.dt.float32
BF16 = mybir.dt.bfloat16
X = mybir.AxisListType


class _TileContext(tile.TileContext):
    """Workaround: this walrus build allows only 1 sync-wait per
    instruction; stock TileContext packs one wait per outstanding proc
    onto the single tail drain. Spread them across multiple drains."""

    def _drain_and_barrier(self, tick_clock, wait_clock):
        nc = self.nc
        drain_inst = nc.sync.drain()
        wait_clock.add_sem_waits(
            drain_inst.ins, ScopedClock({None: tick_clock.global_clock})
        )
        si = drain_inst.ins.sync_info
        waits = list(si.on_wait) if si is not None and si.on_wait else []
        if len(waits) > 1:
            drain_inst.ins.sync_info = mybir.SyncInfo(
                on_wait=[waits[0]], on_update=list(si.on_update or [])
            )
            for w in waits[1:]:
                d = nc.sync.drain()
                d.ins.sync_info = mybir.SyncInfo(on_wait=[w], on_update=[])
        nc.all_engine_barrier()
        assert self.sems is not None
        popped = nc._tile_sem_poison_stack.pop()
        assert popped is self._sem_poison
        nc.clear_and_free_semaphores(list(self.sems.allocated().values()))
        nc.all_engine_barrier()


def _split_multi_waits(nc):
    """This walrus build accepts only 1 sync-wait per instruction (2 for
    EventSemaphore). Hoist extra semaphore waits onto same-engine NOPs
    inserted immediately before the instruction (engine queues are strict
    FIFO, so a preceding wait-NOP is semantically identical)."""
    for fn in nc.m.functions:
        for blk in fn.blocks:
            insts = blk.instructions
            out = []
            for inst in insts:
                si = inst.sync_info
                waits = list(si.on_wait) if si is not None and si.on_wait else []
                cap = 2 if isinstance(inst, mybir.InstEventSemaphore) else 1
                if len(waits) > cap and inst.engine != mybir.EngineType.Unassigned:
                    for w in waits[:-1]:
                        nop = mybir.InstNoOp(
                            name=f"{inst.name}-w{len(out)}",
                            engine=inst.engine,
                            sync_info=mybir.SyncInfo(on_wait=[w], on_update=[]),
                            bass_nofuse=True,
                        )
                        nc.register_instruction(nop, overwrite=True)
                        out.append(nop)
                    inst.sync_info = mybir.SyncInfo(
                        on_wait=[waits[-1]], on_update=list(si.on_update or [])
                    )
                out.append(inst)
            blk.instructions = out


def build_kernel(
    repeat=1, n_tiles=NT, stages=("mul", "trans", "scat", "stats"), loop=None
):
    """Build the per-core Bass module. repeat: replicate the whole pass
    unrolled. loop: wrap the pass in a hardware For_i loop executing it
    `loop` times on-device (timing builds; outputs stay valid because the
    accumulators are overwritten, not accumulated, each pass).
    stages: knock out pipeline stages for profiling."""
    ng = max(1, n_tiles // G)
    nc = bass.Bass("TRN2", target_bir_lowering=False, debug=False, num_devices=1)
    pred_d = nc.dram_tensor("pred", [ROWS, T], F32, kind="ExternalInput")
    dem_d = nc.dram_tensor("dem", [ROWS, D], F32, kind="ExternalInput")
    mask_d = nc.dram_tensor("mask", [P, NCH * RR * L], BF16, kind="ExternalInput")
    ident_d = nc.dram_tensor("ident", [P, P], BF16, kind="ExternalInput")
    rrep_d = nc.dram_tensor("rrep", [P, L], F32, kind="ExternalInput")
    out_d = nc.dram_tensor("partials", [3, P, ng], F32, kind="ExternalOutput")

    with _TileContext(nc) as tc:
        with ExitStack() as ctx:
            singles = ctx.enter_context(tc.tile_pool(name="singles", bufs=1))
            io = ctx.enter_context(tc.tile_pool(name="io", bufs=2))
            work = ctx.enter_context(tc.tile_pool(name="work", bufs=3))
            evac = ctx.enter_context(tc.tile_pool(name="evac", bufs=3))
            small = ctx.enter_context(tc.tile_pool(name="small", bufs=2))
            tpsA = ctx.enter_context(tc.tile_pool(name="tpsA", bufs=2, space="PSUM"))
            tpsB = ctx.enter_context(tc.tile_pool(name="tpsB", bufs=2, space="PSUM"))
            upsum = ctx.enter_context(tc.tile_pool(name="upsum", bufs=2, space="PSUM"))

            ident_t = singles.tile([P, P], BF16)
            nc.sync.dma_start(ident_t[:], ident_d.ap())
            mask_t = singles.tile([P, NCH, RR * L], BF16)
            nc.sync.dma_start(
                mask_t[:], mask_d.ap().rearrange("p (c m) -> p c m", c=NCH)
            )
            rrep_t = singles.tile([P, L], F32)
            nc.sync.dma_start(rrep_t[:], rrep_d.ap())
            accq = singles.tile([P, ng], F32)
            accs2 = singles.tile([P, ng], F32)
            accm = singles.tile([P, ng], F32)
            if "stats" not in stages:  # profiling builds: keep outputs defined
                for acc in (accq, accs2, accm):
                    nc.gpsimd.memset(acc[:], 0.0)

            loop_cm = tc.For_i(0, loop, 1) if loop is not None else None
            if loop_cm is not None:
                loop_cm.__enter__()
            for rep in range(repeat):
                for g in range(ng):
                    u_ps = upsum.tile([P, G * RR, L], F32)
                    # one big DMA per group: 4 tiles of pred + dem
                    gi = g % (n_tiles // G)
                    pred_g = io.tile([P, G, TW], F32)
                    nc.sync.dma_start(
                        pred_g[:],
                        pred_d.ap()[gi * GROWS : (gi + 1) * GROWS, :].rearrange(
                            "(r p rr) t -> p r (rr t)", p=P, rr=RR
                        ),
                    )
                    dem_g = io.tile([P, G, RR * D], F32)
                    nc.sync.dma_start(
                        dem_g[:],
                        dem_d.ap()[gi * GROWS : (gi + 1) * GROWS, :].rearrange(
                            "(r p rr) d -> p r (rr d)", p=P, rr=RR
                        ),
                    )
                    for j in range(G):
                        if "mul" not in stages:
                            continue
                        tt = work.tile([P, TWPAD], BF16)
                        nc.gpsimd.memset(tt[:, TW:TWPAD], 0.0)
                        for rr in range(RR):
                            nc.vector.tensor_tensor(
                                out=tt[:, rr * T : (rr + 1) * T].rearrange(
                                    "p (d j) -> p d j", j=8
                                ),
                                in0=pred_g[:, j, rr * T : (rr + 1) * T].rearrange(
                                    "p (d j) -> p d j", j=8
                                ),
                                in1=dem_g[:, j, rr * D : (rr + 1) * D]
                                .unsqueeze(2)
                                .broadcast_to([P, D, 8]),
                                op=mybir.AluOpType.mult,
                            )
                        if "trans" not in stages:
                            continue
                        ttA_ps = tpsA.tile([P, NCA, P], BF16)
                        ttB_ps = tpsB.tile([P, NCB, P], BF16)
                        for c in range(NCH):
                            dst = (
                                ttA_ps[:, c, :] if c < NCA else ttB_ps[:, c - NCA, :]
                            )
                            nc.tensor.transpose(
                                out=dst,
                                in_=tt[:, c * P : (c + 1) * P],
                                identity=ident_t[:],
                            )
                        ttA = evac.tile([P, NCA, P], BF16)
                        nc.scalar.copy(out=ttA[:], in_=ttA_ps[:])
                        ttB = evac.tile([P, NCB, P], BF16)
                        nc.scalar.copy(out=ttB[:], in_=ttB_ps[:])
                        if "scat" not in stages:
                            continue
                        for c in range(NCH):
                            src = ttA[:, c, :] if c < NCA else ttB[:, c - NCA, :]
                            nc.tensor.matmul(
                                out=u_ps[:, RR * j : RR * (j + 1), :],
                                lhsT=src,
                                rhs=mask_t[:, c, :],
                                start=(c == 0),
                                stop=(c == NCH - 1),
                            )
                    # --- stats for this group of G tiles (G*RR row-units) ---
                    if "stats" not in stages:
                        continue
                    u_sb = work.tile([P, G * RR, L], F32)
                    nc.vector.tensor_tensor(
                        out=u_sb[:],
                        in0=u_ps[:],
                        in1=rrep_t[:].unsqueeze(1).broadcast_to([P, G * RR, L]),
                        op=mybir.AluOpType.mult,
                    )
                    s8 = small.tile([P, G * RR], F32)
                    nc.vector.reduce_sum(out=s8[:], in_=u_sb[:], axis=X.X)
                    m8 = small.tile([P, G * RR], F32)
                    nc.vector.reduce_max(out=m8[:], in_=u_sb[:], axis=X.X)
                    usq = work.tile([P, G * RR, L], F32)
                    nc.vector.tensor_tensor(
                        out=usq[:],
                        in0=u_sb[:],
                        in1=u_sb[:],
                        op=mybir.AluOpType.mult,
                    )
                    nc.vector.reduce_sum(out=accq[:, g : g + 1], in_=usq[:], axis=X.XY)
                    s2s = small.tile([P, G * RR], F32)
                    nc.vector.tensor_tensor(
                        out=s2s[:],
                        in0=s8[:],
                        in1=s8[:],
                        op=mybir.AluOpType.mult,
                    )
                    nc.vector.reduce_sum(out=accs2[:, g : g + 1], in_=s2s[:], axis=X.X)
                    nc.vector.reduce_sum(out=accm[:, g : g + 1], in_=m8[:], axis=X.X)
            if loop_cm is not None:
                loop_cm.__exit__(None, None, None)
            nc.sync.dma_start(out_d.ap()[0], accq[:])
            nc.sync.dma_start(out_d.ap()[1], accs2[:])
            nc.sync.dma_start(out_d.ap()[2], accm[:])
    _split_multi_waits(nc)
    return nc


def make_constants(tunnel_to_link, link_capacities):
    t2l = np.asarray(tunnel_to_link).astype(np.int64).ravel()
    cap = np.asarray(link_capacities, dtype=np.float32).ravel()
    # mask[k, c, rr*L + l]: chunk c covers padded free idx f = c*128 + k,
    # f = rr*T + t (f < TW); one-hot into (rr, link(t)); pad rows zero.
    mask = np.zeros((P, NCH, RR * L), dtype=np.float32)
    for f in range(TW):
        c, k = divmod(f, P)
        rr, t = divmod(f, T)
        mask[k, c, rr * L + int(t2l[t])] = 1.0
    mask = mask.reshape(P, NCH * RR * L)
    ident = np.eye(P, dtype=np.float32)
    rrep = np.broadcast_to(
        (1.0 / (cap + 1e-8)).astype(np.float32)[None, :], (P, L)
    ).copy()
    return mask.astype(np.float32), ident.astype(np.float32), rrep


def _to_bf16(a):
    # numpy has no bf16; round-to-nearest-even via ml_dtypes if present,
    # else truncate+round manually and keep uint16 view.
    try:
        import ml_dtypes

        return a.astype(ml_dtypes.bfloat16)
    except ImportError:
        x = a.astype(np.float32).view(np.uint32)
        x = (x + 0x7FFF + ((x >> 16) & 1)) >> 16
        return x.astype(np.uint16)


def run_cores(nc, pred, dem, mask, ident, rrep, **kw):
    pred = np.ascontiguousarray(np.asarray(pred, dtype=np.float32))
    dem = np.ascontiguousarray(np.asarray(dem, dtype=np.float32))
    mask_bf = _to_bf16(mask)
    ident_bf = _to_bf16(ident)
    in_maps = []
    for i in range(N_CORES):
        in_maps.append(
            {
                "pred": pred[i * ROWS : (i + 1) * ROWS],
                "dem": dem[i * ROWS : (i + 1) * ROWS],
                "mask": mask_bf,
                "ident": ident_bf,
                "rrep": rrep,
            }
        )
    return run_bass_kernel_spmd(nc, in_maps, core_ids=list(range(N_CORES)), **kw)


def combine_partials(partials_list):
    q = s2 = m = 0.0
    for p in partials_list:
        p = np.asarray(p, dtype=np.float64)
        q += p[0].sum()
        s2 += p[1].sum()
        m += p[2].sum()
    var_mean = (q - s2 / L) / (L - 1) / B
    return var_mean + 0.5 * m / B


def kernel(pred_ratios, demands, tunnel_to_link, link_capacities):
    mask, ident, rrep = make_constants(tunnel_to_link, link_capacities)
    nc = build_kernel()
    res = run_cores(nc, pred_ratios, demands, mask, ident, rrep)
    loss = combine_partials([r["partials"] for r in res.results])
    return np.array(loss, dtype=np.float32)

